# revision 3
# baseline (speedup 1.0000x reference)
"""AAM (additive angular margin) loss on 8 TRN2 NeuronCores.

loss = mean_r [ logsumexp_c(30 * (x_hat[r,c] - 0.5*onehot(label_r))) - 30*(x_hat[r,label_r] - 0.5) ]
with x_hat = x / max(||x||_2, 1e-12) per row.

Strategy: shard rows across 8 cores (1024 rows each). Each core streams its
[1024, 32000] f32 shard from HBM exactly once (8 row-blocks of 128 partitions,
each split into 8 column chunks that stay resident in SBUF between the two
passes):
  pass 1 (VectorE): ss = sum(x^2) per row  (tensor_tensor_reduce)
  ACT:  inv-scale = 30/sqrt(ss) computed as exp(-0.5*ln(ss) + ln 30)
        (ln and exp share one ACT table set - no table switches)
  pass 2 (ScalarE): S = sum(exp(scale * x)) per row, in-place, accum_out
The margin term needs only x[r, label_r], gathered once per core with a
1024-element indirect DMA; the label column of the softmax sum is corrected
analytically: S' = S - exp(30t) + exp(30t - 15), t = x_label/||x||.
nll = ln(S') - (30t - 15).  Per-core sum via a [128,1]x[128,1] matmul against
a 1/N vector, then an 8-core AllReduce of the scalar; every core writes the
final mean.
"""

import math

import numpy as np

MARGIN = 0.5
SCALE = 30.0
N_CORES = 8
N_TOTAL = 8192
C = 32000
P = 128

R = N_TOTAL // N_CORES  # rows per core
B = R // P  # row blocks per core
CK = 4000  # column chunk
NCHUNK = C // CK
CHUNK_BUFS = 11


def build(n_rows=R, n_cols=C, ck=CK, n_cores=N_CORES, n_total=N_TOTAL):
    """Build + compile the per-core Bass graph (SPMD, identical on all cores)."""
    import concourse.bacc as bacc
    import concourse.bass as bass
    import concourse.tile as tile
    from concourse import mybir

    f32 = mybir.dt.float32
    u32 = mybir.dt.uint32
    AF = mybir.ActivationFunctionType
    ALU = mybir.AluOpType
    AX = mybir.AxisListType

    b_blocks = n_rows // P
    nchunk = n_cols // ck
    assert n_rows % P == 0 and n_cols % ck == 0

    nc = bacc.Bacc("TRN2", target_bir_lowering=False, debug=False, num_devices=n_cores)

    logits_ext = nc.dram_tensor("logits", [n_rows, n_cols], f32, kind="ExternalInput")
    goff_ext = nc.dram_tensor("goff", [P, b_blocks], u32, kind="ExternalInput")
    out_ext = nc.dram_tensor("out", [1, 1], f32, kind="ExternalOutput")

    neg_m = -SCALE * MARGIN  # -15
    ln_s = math.log(SCALE)

    with tile.TileContext(nc) as tc:
        with (
            tc.tile_pool(name="chunks", bufs=CHUNK_BUFS) as chunks,
            tc.tile_pool(name="singles", bufs=1) as singles,
            tc.tile_pool(name="smalls", bufs=3) as smalls,
            tc.tile_pool(name="ppool", bufs=1, space="PSUM") as ppool,
            tc.tile_pool(name="dpool", bufs=1, space="DRAM") as dpool,
        ):
            # label-logit gather: one indirect DMA for all rows of this core
            goff_sb = singles.tile([P, b_blocks], u32)
            nc.sync.dma_start(out=goff_sb[:, :], in_=goff_ext[:, :])
            xl_all = singles.tile([P, b_blocks], f32)
            logits_flat = logits_ext.ap().rearrange("r (c one) -> (r c) one", one=1)
            nc.gpsimd.indirect_dma_start(
                out=xl_all[:, :],
                out_offset=None,
                in_=logits_flat,
                in_offset=bass.IndirectOffsetOnAxis(ap=goff_sb[:, :], axis=0),
            )

            zero_t = singles.tile([P, 1], f32)
            nc.vector.memset(zero_t, 0.0)
            m15_t = singles.tile([P, 1], f32)
            nc.vector.memset(m15_t, neg_m)
            ln30_t = singles.tile([P, 1], f32)
            nc.vector.memset(ln30_t, ln_s)
            invn_t = singles.tile([P, 1], f32)
            nc.vector.memset(invn_t, 1.0 / n_total)

            nll_all = singles.tile([P, b_blocks], f32)
            dump = singles.tile([P, ck], f32)

            for b in range(b_blocks):
                ss_cols = smalls.tile([P, nchunk], f32, tag="ss_cols")
                es_cols = smalls.tile([P, nchunk], f32, tag="es_cols")
                chs = []
                for c in range(nchunk):
                    ch = chunks.tile([P, ck], f32, tag="chunk", name=f"ch_{b}_{c}")
                    nc.sync.dma_start(
                        out=ch[:, :],
                        in_=logits_ext[b * P : (b + 1) * P, c * ck : (c + 1) * ck],
                    )
                    # ss_cols[:, c] = sum(ch * ch) along free dim
                    nc.vector.scalar_tensor_tensor(
                        out=dump[:, :],
                        in0=ch[:, :],
                        scalar=1.0,
                        in1=ch[:, :],
                        op0=ALU.mult,
                        op1=ALU.mult,
                        accum_out=ss_cols[:, c : c + 1],
                    )
                    chs.append(ch)

                ss = smalls.tile([P, 1], f32, tag="ss")
                nc.vector.reduce_sum(out=ss[:, :], in_=ss_cols[:, :], axis=AX.X)
                ssc = smalls.tile([P, 1], f32, tag="ssc")
                # clamp: max(ss, eps^2) so 1/sqrt matches x/max(||x||, eps)
                nc.vector.tensor_scalar_max(out=ssc[:, :], in0=ss[:, :], scalar1=1e-24)
                u = smalls.tile([P, 1], f32, tag="u")
                nc.scalar.activation(out=u[:, :], in_=ssc[:, :], func=AF.Ln, bias=zero_t[:, :])
                # sca = 30 / sqrt(ssc) = exp(-0.5*ln(ssc) + ln(30))
                sca = smalls.tile([P, 1], f32, tag="sca")
                nc.scalar.activation(
                    out=sca[:, :], in_=u[:, :], func=AF.Exp, bias=ln30_t[:, :], scale=-0.5
                )
                # t30 = 30 * x_label / ||x||
                t30 = smalls.tile([P, 1], f32, tag="t30")
                nc.vector.tensor_tensor(
                    out=t30[:, :], in0=xl_all[:, b : b + 1], in1=sca[:, :], op=ALU.mult
                )
                e1 = smalls.tile([P, 1], f32, tag="e1")
                nc.scalar.activation(out=e1[:, :], in_=t30[:, :], func=AF.Exp, bias=zero_t[:, :])
                e2 = smalls.tile([P, 1], f32, tag="e2")
                nc.scalar.activation(out=e2[:, :], in_=t30[:, :], func=AF.Exp, bias=m15_t[:, :])

                # pass 2: es_cols[:, c] = sum(exp(sca * x)) along free dim, in place
                for c, ch in enumerate(chs):
                    nc.scalar.activation(
                        out=ch[:, :],
                        in_=ch[:, :],
                        func=AF.Exp,
                        bias=zero_t[:, :],
                        scale=sca[:, 0:1],
                        accum_out=es_cols[:, c : c + 1],
                    )

                s_sum = smalls.tile([P, 1], f32, tag="s_sum")
                nc.vector.reduce_sum(out=s_sum[:, :], in_=es_cols[:, :], axis=AX.X)
                # sc2 = s_sum - e1 + e2  (replace label term with margined one)
                sc1 = smalls.tile([P, 1], f32, tag="sc1")
                nc.vector.scalar_tensor_tensor(
                    out=sc1[:, :],
                    in0=e1[:, :],
                    scalar=-1.0,
                    in1=s_sum[:, :],
                    op0=ALU.mult,
                    op1=ALU.add,
                )
                sc2 = smalls.tile([P, 1], f32, tag="sc2")
                nc.vector.tensor_tensor(out=sc2[:, :], in0=sc1[:, :], in1=e2[:, :], op=ALU.add)
                lse = smalls.tile([P, 1], f32, tag="lse")
                nc.scalar.activation(out=lse[:, :], in_=sc2[:, :], func=AF.Ln, bias=zero_t[:, :])
                # nll = lse - (t30 - 15) = (lse + 15) - t30
                nc.vector.scalar_tensor_tensor(
                    out=nll_all[:, b : b + 1],
                    in0=lse[:, :],
                    scalar=-neg_m,
                    in1=t30[:, :],
                    op0=ALU.add,
                    op1=ALU.subtract,
                )

            # per-core scalar: sum_p sum_b nll / n_total  (partition reduce by matmul)
            nll_row = singles.tile([P, 1], f32)
            nc.vector.reduce_sum(out=nll_row[:, :], in_=nll_all[:, :], axis=AX.X)
            pt = ppool.tile([1, 1], f32)
            nc.tensor.matmul(
                out=pt[:, :], lhsT=nll_row[:, :], rhs=invn_t[:, :], start=True, stop=True
            )
            final_sb = singles.tile([1, 1], f32)
            nc.vector.tensor_copy(out=final_sb[:, :], in_=pt[:, :])

            ar_in = dpool.tile([1, 1], f32)
            ar_out = dpool.tile([1, 1], f32, addr_space="Shared")
            nc.sync.dma_start(out=ar_in[:, :], in_=final_sb[:, :])
            nc.gpsimd.collective_compute(
                "AllReduce",
                mybir.AluOpType.add,
                replica_groups=[list(range(n_cores))],
                ins=[ar_in.opt()],
                outs=[ar_out.opt()],
            )
            nc.gpsimd.dma_start(out=out_ext[:, :], in_=ar_out[:, :])

    nc.compile()
    return nc


_NC_CACHE = None


def _get_nc():
    global _NC_CACHE
    if _NC_CACHE is None:
        _NC_CACHE = build()
    return _NC_CACHE


def make_in_maps(logits, labels):
    logits = np.ascontiguousarray(np.asarray(logits, dtype=np.float32))
    labels = np.asarray(labels).astype(np.int64)
    assert logits.shape == (N_TOTAL, C), logits.shape
    in_maps = []
    for i in range(N_CORES):
        shard = logits[i * R : (i + 1) * R]
        lab = labels[i * R : (i + 1) * R]
        flat = np.arange(R, dtype=np.int64) * C + lab  # local flat element index
        goff = np.ascontiguousarray(flat.reshape(B, P).T).astype(np.uint32)
        in_maps.append({"logits": shard, "goff": goff})
    return in_maps


def kernel(**inputs):
    from concourse.bass_utils import run_bass_kernel_spmd

    nc = _get_nc()
    in_maps = make_in_maps(inputs["logits"], inputs["labels"])
    res = run_bass_kernel_spmd(nc, in_maps, core_ids=list(range(N_CORES)))
    out = np.asarray(res.results[0]["out"], dtype=np.float32)
    return out.reshape(())
